# revision 1
# baseline (speedup 1.0000x reference)
"""Trainium2 Bass kernel for nn_InverseResNet (dense MLP with fixed-point blocks).

Reference computation (per row of x):
  h = x @ W_init + b_init                       # [128] -> [256]
  for b in 4 blocks:  (y = h)
      repeat 10: x <- y - (relu(x @ Wg1[b] + bg1[b]) @ Wg2[b] + bg2[b])
      h = x
  out = h @ W_final + b_final                   # [256] -> [128]

Mathematical restructuring (all host-side, float64):
  * Track t_k = relu(x_k W1 + b1) instead of x_k:
        t_{k+1} = relu(c + t_k @ mn),  mn = -(W2 @ W1),  c = y W1 + (b1 - b2 W1)
    so each inner iteration is ONE composed 256x256 matmul instead of two.
  * The reference's 10 iterations are converged far below the 2e-2 gate:
    NI=3 g-evaluations reproduce the reference to ~8e-3 absmax-rel on
    hardware (NI=4: ~5e-3), fp8 noise included.
  * mn and the iteration activations are fp8e4m3; the tensor engine runs the
    composed matmul in DoubleRow perf mode (the full 256-contraction in one
    instruction at 0.5 cycles/row).  c stays f32r; the final iterate t is
    materialized in f32r, so fp8 noise reaching the output is one damped step.
  * b_init / bg2 / b_final shifts are never applied on-chip: they fold into
    the per-block biases e'' and b1'' and the final bias (cumulative fold).
  * W_init is composed into block 0's head (w10 = Wi @ W1_0) and block 3's
    tail into the final layer (wf_t = -(W2_3 @ Wf)).
  * No c tensor is ever materialized (RECOMP mode): iteration 1 accumulates
    its fp8 DoubleRow matmul directly onto the head's PSUM group (stop is a
    sim-only flag; hardware accumulation keys off the start bit), and
    iteration 2 re-runs the cheap W1 matmuls into its own group, with e''
    applied as the relu bias.  This removes 8 element-wise ops and all
    identity-inject matmuls per tile.

Strategy: pure data parallel over 8 NeuronCores (batch 65536 -> 8192 rows/core,
16 batch tiles of 512 columns each).  Activations live feature-major
[128 features, 2 groups, 512 cols]; the host passes x pre-transposed and
re-transposes y, so no on-device transposes at all.  Every PSUM tile is a
single bank [128, 512] rotating through all 8 banks; c is injected into each
iteration's accumulation group by an f32r identity matmul (start=True) and the
fp8 DoubleRow matmul lands on top.  Tiles are software-pipelined (a new tile
enters the stage machine every STAGGER stages) so PE-heavy iteration stages
and ACT/DVE-heavy head/tail stages of different tiles overlap; element-wise
ops are greedily balanced across ACT and DVE (GPSIMD cannot touch PSUM and
has no ucode for these ops, so it idles).  Steady state is PE-bound at 100%
occupancy with ACT/DVE at ~85%.  Block 2 additionally runs in delta form
(q2 = q1 + mn(t1-t0) accumulated in place, e'' injected once via a ones-row
DoubleRow with split-fp8 planes) — applying it to more blocks exhausts the
8 PSUM banks' residency and loses.  TimelineSim: 204.3 us vs 976 us baseline
(4.76x); hardware-verified absmax-rel 8.4e-3 (gate 2e-2).
"""

import os
import numpy as np

N_CORES = 8
BATCH, LATENT, HIDDEN, OUT = 65536, 128, 256, 128
NBLOCKS = 4
B_CORE = BATCH // N_CORES      # 8192
TILE_N = 512                   # batch columns per matmul (1 PSUM bank of fp32)
N_TILES = B_CORE // TILE_N     # 16
PAIR = int(os.environ.get("KERNEL_PAIR", 4))   # batch tiles in flight
NI = int(os.environ.get("KERNEL_NI", 3))       # g-evaluations per block
FP8 = int(os.environ.get("KERNEL_FP8", 1))     # fp8 DoubleRow iterations
UNIFY = int(os.environ.get("KERNEL_UNIFY_PSUM", 1))   # single 4-slot psum tag
SPLITR = int(os.environ.get("KERNEL_SPLIT_RELU", 0))  # iter relu as 2 half ops
SPLITT = int(os.environ.get("KERNEL_SPLIT_TAIL", 0))  # tail add as 2 half ops
ENGMODE = int(os.environ.get("KERNEL_ENGMODE", 0))    # 0 rotate all; 1 relu->ACT
STAGGER = int(os.environ.get("KERNEL_STAGGER", 3))    # stages between tile starts
PERM = int(os.environ.get("KERNEL_PERM", 1))          # per-half 1-bank psum slots
INJ8 = int(os.environ.get("KERNEL_INJ8", 0))          # fp8 DR c-inject (else f32r)
ITFUSE = int(os.environ.get("KERNEL_ITFUSE", 0))      # fused 2-bank iter psum+relu
REUSE = int(os.environ.get("KERNEL_REUSE", 0))        # iter-1 accumulates on head psum
RECOMP = int(os.environ.get("KERNEL_RECOMP", 1))      # c-free: recompute W1 per iter
DELTA = int(os.environ.get("KERNEL_DELTA", 4))        # delta-form iterations

_CACHE = {}


def _build(n_tiles=N_TILES, ni=NI, fp8=FP8, pair=PAIR, unify=None, splitr=None, splitt=None, engmode=None, stagger=None, perm=None, inj8=None, itfuse=None, reuse=None, recomp=None, delta=None):
    delta = DELTA if delta is None else delta
    recomp = RECOMP if recomp is None else recomp
    reuse = REUSE if reuse is None else reuse
    itfuse = ITFUSE if itfuse is None else itfuse
    inj8 = INJ8 if inj8 is None else inj8
    perm = PERM if perm is None else perm
    stagger = STAGGER if stagger is None else stagger
    engmode = ENGMODE if engmode is None else engmode
    unify = UNIFY if unify is None else unify
    splitr = SPLITR if splitr is None else splitr
    splitt = SPLITT if splitt is None else splitt
    from contextlib import ExitStack
    import concourse.bacc as bacc
    import concourse.tile as tile
    import concourse.mybir as mybir
    from concourse.masks import make_identity
    from concourse.alu_op_type import AluOpType

    f32 = mybir.dt.float32
    f32r = mybir.dt.float32r
    f8 = mybir.dt.float8e4
    AF = mybir.ActivationFunctionType
    DR = mybir.MatmulPerfMode.DoubleRow
    it_dt = f8 if fp8 else f32r

    nc = bacc.Bacc("TRN2", target_bir_lowering=False, debug=False,
                   num_devices=N_CORES)

    x_d = nc.dram_tensor("x", [LATENT, B_CORE], f32, kind="ExternalInput").ap()
    w10_d = nc.dram_tensor("w10", [128, HIDDEN], f32, kind="ExternalInput").ap()
    w1_d = nc.dram_tensor("w1", [128, NBLOCKS, 2, HIDDEN], f32, kind="ExternalInput").ap()
    mn_d = nc.dram_tensor("mn", [128, NBLOCKS, 2, HIDDEN], it_dt, kind="ExternalInput").ap()
    w2_d = nc.dram_tensor("w2", [128, NBLOCKS, 2, HIDDEN], f32, kind="ExternalInput").ap()
    wi_d = nc.dram_tensor("wi", [128, HIDDEN], f32, kind="ExternalInput").ap()
    wfh_d = nc.dram_tensor("wfh", [128, 2, OUT], f32, kind="ExternalInput").ap()
    wft_d = nc.dram_tensor("wft", [128, 2, OUT], f32, kind="ExternalInput").ap()
    e_d = nc.dram_tensor("e", [128, NBLOCKS, 2], f32, kind="ExternalInput").ap()
    b1_d = nc.dram_tensor("b1", [128, NBLOCKS, 2], f32, kind="ExternalInput").ap()
    bf_d = nc.dram_tensor("bf", [128, 1], f32, kind="ExternalInput").ap()
    id8_d = nc.dram_tensor("id8", [128, 2, 128], it_dt, kind="ExternalInput").ap()
    d_d = nc.dram_tensor("d", [128, NBLOCKS, 2], f32, kind="ExternalInput").ap()
    if delta:
        e8p_d = nc.dram_tensor("e8p", [1, NBLOCKS, 2, 2, 128], it_dt, kind="ExternalInput").ap()
        ones8_d = nc.dram_tensor("ones8", [1, 2, TILE_N], it_dt, kind="ExternalInput").ap()
    y_d = nc.dram_tensor("y", [OUT, B_CORE], f32, kind="ExternalOutput").ap()

    def r(ap):
        return ap.bitcast(f32r)

    with tile.TileContext(nc) as tc, ExitStack() as ctx:
        wp = ctx.enter_context(tc.tile_pool(name="weights", bufs=1))
        n_stages_est = 2 + NBLOCKS * (ni + 1) - 1
        inflight = (max((n_stages_est + stagger - 1) // stagger, 2) + 1
                    if stagger else pair)
        BUFS = {
            "c": inflight + 1, "h": inflight + 1, "tf": inflight + 1,
            "t8": 2 * inflight,
            "xin": 3, "xt": (6 // max(stagger, 1)) + 2 if stagger else pair + 1,
            "ot": 3, "on": 3,
        } if stagger else {
            "c": 2 * pair, "h": 2 * pair, "tf": 2 * pair, "t8": 2 * pair,
            "xin": pair + 1, "xt": pair + 1, "ot": pair + 1, "on": pair + 1,
        }
        hp = ctx.enter_context(tc.tile_pool(name="acts", bufs=1))
        iop = ctx.enter_context(tc.tile_pool(name="io", bufs=1))
        pp = ctx.enter_context(tc.tile_pool(name="psum", bufs=1, space="PSUM"))

        ident = wp.tile([128, 128], f32)
        make_identity(nc, ident)
        identr = wp.tile([128, 128], f32r)
        nc.vector.tensor_copy(out=identr, in_=ident)
        zeros = wp.tile([128, 2, TILE_N], f32)
        nc.vector.memset(zeros, 0.0)

        w10_s = wp.tile([128, HIDDEN], f32r)
        nc.sync.dma_start(out=w10_s, in_=r(w10_d))
        w1_s = wp.tile([128, NBLOCKS, 2, HIDDEN], f32r)
        nc.sync.dma_start(out=w1_s, in_=r(w1_d))
        mn_s = wp.tile([128, NBLOCKS, 2, HIDDEN], it_dt)
        nc.sync.dma_start(out=mn_s, in_=mn_d if fp8 else r(mn_d))
        w2_s = wp.tile([128, NBLOCKS, 2, HIDDEN], f32r)
        nc.sync.dma_start(out=w2_s, in_=r(w2_d))
        wi_s = wp.tile([128, HIDDEN], f32r)
        nc.sync.dma_start(out=wi_s, in_=r(wi_d))
        wfh_s = wp.tile([128, 2, OUT], f32r)
        nc.sync.dma_start(out=wfh_s, in_=r(wfh_d))
        wft_s = wp.tile([128, 2, OUT], f32r)
        nc.sync.dma_start(out=wft_s, in_=r(wft_d))
        e_s = wp.tile([128, NBLOCKS, 2], f32)
        nc.sync.dma_start(out=e_s, in_=e_d)
        b1_s = wp.tile([128, NBLOCKS, 2], f32)
        nc.sync.dma_start(out=b1_s, in_=b1_d)
        bf_s = wp.tile([128, 1], f32)
        nc.sync.dma_start(out=bf_s, in_=bf_d)
        id8_s = wp.tile([128, 2, 128], it_dt)
        nc.sync.dma_start(out=id8_s, in_=id8_d)
        d_s = wp.tile([128, NBLOCKS, 2], f32)
        nc.sync.dma_start(out=d_s, in_=d_d)
        if delta:
            e8p_s = wp.tile([128, NBLOCKS, 2, 2, 128], it_dt, padded_shape=None)
            nc.sync.dma_start(out=e8p_s[0:1], in_=e8p_d)
            ones8_s = wp.tile([128, 2, TILE_N], it_dt)
            nc.sync.dma_start(out=ones8_s[0:1], in_=ones8_d)

        BIG = dict(tag="big", bufs=4) if unify else dict(tag="big", bufs=3)
        SMALL = dict(tag="big", bufs=4) if unify else dict(tag="small", bufs=2)



        # --- element-wise op rotation across ACT / DVE / Pool -------------
        rot = {"i": 0}
        pats = {
            0: ["act", "dve", "act", "pool", "dve", "act"],
            1: ["act", "dve", "pool", "act", "dve", "pool", "act", "dve",
                "act", "dve", "act"],
            2: ["act", "dve", "act", "dve", "pool", "act", "dve"],
            3: ["act", "dve"],
        }
        ROT = int(os.environ.get("KERNEL_ROT", 9))
        ENGS = pats.get(ROT, pats[0])
        load = {"act": 0.0, "dve": 0.0, "pool": 0.0}
        ECOST = {"act": (612, 185), "dve": (658, 125), "pool": (800, 95)}

        def next_eng(allow_act=True, cols=512, allow_pool=False):
            # GPSIMD cannot access PSUM on hardware; ops with PSUM operands
            # must run on ACT/DVE (allow_pool=False).
            if ROT == 9:  # greedy balance by accumulated cost
                best, bc = None, None
                for e in ("act", "dve", "pool"):
                    if not allow_act and e == "act":
                        continue
                    if not allow_pool and e == "pool":
                        continue
                    base, init = ECOST[e]
                    c = base * cols / 512.0 + init
                    tot = load[e] + c
                    if bc is None or tot < bc:
                        best, bc = e, tot
                base, init = ECOST[best]
                load[best] += base * cols / 512.0 + init
                return best
            while True:
                e = ENGS[rot["i"] % len(ENGS)]
                rot["i"] += 1
                if (allow_act or e != "act") and (allow_pool or e != "pool"):
                    return e

        def op_bias(out, in_, bias, relu, eng=None):
            """out = [relu](in_ + bias); bias is a [128,1] AP or 0.0."""
            ncols = 1
            for dim in out.shape[1:]:
                ncols *= dim
            if eng is None:
                eng = next_eng(allow_act=(engmode == 0), cols=ncols)
            if eng == "act":
                nc.scalar.activation(out=out, in_=in_,
                                     func=AF.Relu if relu else AF.Identity,
                                     bias=bias, scale=1.0)
                return
            v = nc.vector if eng == "dve" else nc.gpsimd
            zs = zeros[:, 0, :out.shape[-1]] if len(out.shape) == 2 else \
                zeros[:, :out.shape[1], :out.shape[-1]]
            if relu:
                v.scalar_tensor_tensor(out=out, in0=in_, scalar=bias, in1=zs,
                                       op0=AluOpType.add, op1=AluOpType.max)
            else:
                v.scalar_tensor_tensor(out=out, in0=in_, scalar=bias, in1=zs,
                                       op0=AluOpType.add, op1=AluOpType.add)

        def op_sub(out, in0, in1):
            """out = in0 - in1 (DVE/Pool only)."""
            eng = next_eng(allow_act=False)
            v = nc.vector if eng == "dve" else nc.gpsimd
            v.scalar_tensor_tensor(out=out, in0=in0, scalar=0.0, in1=in1,
                                   op0=AluOpType.add, op1=AluOpType.subtract)

        def op_add(out, in0, in1):
            """out = in0 + in1 (two full tensors; DVE/Pool only)."""
            eng = next_eng(allow_act=False)
            v = nc.vector if eng == "dve" else nc.gpsimd
            v.scalar_tensor_tensor(out=out, in0=in0, scalar=0.0, in1=in1,
                                   op0=AluOpType.add, op1=AluOpType.add)

        # ------------------------------------------------------------------
        def stage_in(t):
            """Load tile t (x is staged feature-major in DRAM by the host)."""
            xt = iop.tile([128, TILE_N], f32r, tag="xt", bufs=BUFS["xt"])
            nc.sync.dma_start(out=xt, in_=r(x_d[:, t * TILE_N:(t + 1) * TILE_N]))
            return xt

        def psum_m():
            if perm:
                nb = 4 if itfuse else 8
                return [pp.tile([128, TILE_N], f32, tag="ps", bufs=nb,
                                name="ps") for _ in range(2)]
            t = pp.tile([128, 2, TILE_N], f32, **BIG)
            return [t[:, 0, :], t[:, 1, :]]

        def emit_w1(ps_m, m, xtf, h, blk, stop, start=True):
            """Accumulate W1^T y (the head product) into one psum half."""
            if blk == 0:
                nc.tensor.matmul(ps_m, w10_s[:, m * 128:(m + 1) * 128], xtf,
                                 start=start, stop=stop)
            else:
                for kg in range(2):
                    nc.tensor.matmul(
                        ps_m, w1_s[:, blk, kg, m * 128:(m + 1) * 128],
                        h[:, kg, :], start=(start and kg == 0),
                        stop=(stop and kg == 1))

        def emit_head(xtf, h, blk):
            """c = ps + e''; t0 = relu(ps + b1'');  ps = W1^T y."""
            dblk = (delta >> blk) & 1
            ps = psum_m()
            for m in range(2):
                emit_w1(ps[m], m, xtf, h, blk, stop=not dblk)
                if dblk:
                    # inject e'' (two fp8 planes) via a ones-row DoubleRow
                    nc.tensor.matmul(ps[m], e8p_s[0:1, blk, m],
                                     ones8_s[0:1], start=False, stop=True,
                                     perf_mode=DR)
            t8 = hp.tile([128, 2, TILE_N], it_dt, tag="t8", name="t8", bufs=BUFS["t8"])
            if recomp == 1 or (recomp == 2 and blk == 0):
                c = None
            elif fp8 and inj8:
                # c split into two fp8 planes (c8 + cr8 ~ 0.2% precision);
                # e'' is applied exactly as the per-iteration relu bias.
                c = hp.tile([128, 2, 2, TILE_N], it_dt, tag="c", name="c",
                            bufs=BUFS["c"])
                for m in range(2):
                    op_bias(c[:, m, 0, :], ps[m], 0.0, False)
                for m in range(2):
                    op_sub(c[:, m, 1, :], ps[m], c[:, m, 0, :])
            else:
                c = hp.tile([128, 2, TILE_N], f32r, tag="c", name="c",
                            bufs=BUFS["c"])
                for m in range(2):
                    op_bias(c[:, m, :], ps[m], e_s[:, blk, m:m + 1], False)
            tb = d_s if dblk else b1_s
            for m in range(2):
                op_bias(t8[:, m, :], ps[m], tb[:, blk, m:m + 1], True)
            return c, t8, ps

        def emit_iter(c, t8, blk, last, first=False, head_ps=None,
                      xtf=None, h=None):
            """t <- relu(c + mn^T t): inject c (f32r identity matmul), then
            the composed-matrix matmul accumulates on top."""
            dblk = (delta >> blk) & 1
            use_head = bool(
                ((reuse or recomp) and first and head_ps is not None)
                or (dblk and head_ps is not None))
            if itfuse and not use_head:
                big = pp.tile([128, 2, TILE_N], f32, tag="itps", bufs=2,
                              name="itps")
                ps = [big[:, 0, :], big[:, 1, :]]
            else:
                big = None
                ps = head_ps if use_head else psum_m()
            for m in range(2):
                if not use_head and (recomp == 1 or (recomp == 2 and blk == 0)):
                    emit_w1(ps[m], m, xtf, h, blk, stop=False)
                elif not use_head:
                    if fp8 and inj8 and first:
                        nc.tensor.matmul(ps[m], id8_s[:, 0, :], c[:, m, 0, :],
                                         start=True, stop=False)
                    elif fp8 and inj8:
                        nc.tensor.matmul(ps[m], id8_s, c[:, m],
                                         start=True, stop=False, perf_mode=DR)
                    else:
                        nc.tensor.matmul(ps[m], identr, c[:, m, :],
                                         start=True, stop=False)
                if fp8:
                    nc.tensor.matmul(ps[m],
                                     mn_s[:, blk, :, m * 128:(m + 1) * 128],
                                     t8, start=False, stop=True, perf_mode=DR,
                                     skip_group_check=use_head)
                else:
                    for kg in range(2):
                        nc.tensor.matmul(
                            ps[m],
                            mn_s[:, blk, kg, m * 128:(m + 1) * 128],
                            t8[:, kg, :], start=False, stop=(kg == 1),
                            skip_group_check=use_head)
            out = hp.tile([128, 2, TILE_N], f32r if last else it_dt,
                          tag="tf" if last else "t8", name="t",
                          bufs=BUFS["tf"] if last else BUFS["t8"])
            iter_eng = "act" if engmode == 1 else None
            if dblk and not last:
                # d1 = relu(q1) - t0 in one DVE op; next DR applies mn to the
                # increment on the same psum (q2 = q1 + mn d1)
                for m in range(2):
                    nc.vector.scalar_tensor_tensor(
                        out=out[:, m, :], in0=ps[m], scalar=0.0,
                        in1=t8[:, m, :], op0=AluOpType.max,
                        op1=AluOpType.subtract)
                return out
            need_e = ((fp8 and inj8) or use_head
                      or (recomp == 1) or (recomp == 2 and blk == 0))
            if dblk:
                need_e = False
            ibias = ((lambda m: e_s[:, blk, m:m + 1]) if need_e
                     else (lambda m: 0.0))
            if itfuse and not (fp8 and inj8):
                op_bias(out, big, 0.0, True, eng=iter_eng)
            else:
                for m in range(2):
                    op_bias(out[:, m, :], ps[m], ibias(m), True, eng=iter_eng)
            return out

        def emit_tail(xtf, h, tf, blk):
            """h' = y + w2n^T t_final  (w2n = -W2; b2 folded forward)."""
            ps = psum_m()
            for m in range(2):
                for kg in range(2):
                    nc.tensor.matmul(ps[m],
                                     w2_s[:, blk, kg, m * 128:(m + 1) * 128],
                                     tf[:, kg, :],
                                     start=(kg == 0),
                                     stop=(blk != 0 and kg == 1))
                if blk == 0:
                    nc.tensor.matmul(ps[m],
                                     wi_s[:, m * 128:(m + 1) * 128], xtf,
                                     start=False, stop=True)
            hn = hp.tile([128, 2, TILE_N], f32r, tag="h", name="h", bufs=BUFS["h"])
            for m in range(2):
                if blk == 0:
                    op_bias(hn[:, m, :], ps[m], 0.0, False)
                else:
                    op_add(hn[:, m, :], ps[m], h[:, m, :])
            return hn

        def stage_out(t, h, tf):
            """out = Wf^T h3 + (w2n_3 Wf)^T t_final + bf''; transpose; store."""
            ps = pp.tile([128, TILE_N], f32, tag="ps", bufs=(4 if itfuse else 8), name="ps") if perm else pp.tile([128, TILE_N], f32, **SMALL)
            nc.tensor.matmul(ps, wfh_s[:, 0, :], h[:, 0, :],
                             start=True, stop=False)
            nc.tensor.matmul(ps, wfh_s[:, 1, :], h[:, 1, :],
                             start=False, stop=False)
            nc.tensor.matmul(ps, wft_s[:, 0, :], tf[:, 0, :],
                             start=False, stop=False)
            nc.tensor.matmul(ps, wft_s[:, 1, :], tf[:, 1, :],
                             start=False, stop=True)
            ot = iop.tile([128, TILE_N], f32, tag="ot", bufs=BUFS["ot"])
            nc.scalar.activation(out=ot, in_=ps, func=AF.Identity,
                                 bias=bf_s[:, 0:1], scale=1.0)
            nc.sync.dma_start(out=y_d[:, t * TILE_N:(t + 1) * TILE_N], in_=ot)

        if stagger == 0:
            for pair_base in range(0, n_tiles, pair):
                tiles = list(range(pair_base, min(pair_base + pair, n_tiles)))
                xts = [stage_in(t) for t in tiles]
                hs = [None] * len(tiles)
                tfs = [None] * len(tiles)
                for blk in range(NBLOCKS):
                    heads = [emit_head(xts[i], hs[i], blk)
                             for i in range(len(tiles))]
                    cs = [hd[0] for hd in heads]
                    ts = [hd[1] for hd in heads]
                    pss = [hd[2] for hd in heads]
                    for k in range(1, ni):
                        ts = [emit_iter(cs[i], ts[i], blk,
                                        last=(k == ni - 1), first=(k == 1),
                                        head_ps=pss[i], xtf=xts[i], h=hs[i])
                              for i in range(len(tiles))]
                    tfs = ts
                    if blk < NBLOCKS - 1:
                        hs = [emit_tail(xts[i], hs[i], tfs[i], blk)
                              for i in range(len(tiles))]
                for i, t in enumerate(tiles):
                    stage_out(t, hs[i], tfs[i])
        else:
            # software-pipelined: tile t enters the stage machine at step
            # t*stagger, so PE-heavy (iter) and eltwise-heavy (head/tail)
            # stages of different tiles overlap.
            stage_list = [("in", 0)] + [("nop",)] * int(os.environ.get("KERNEL_PF", 1))
            for blk in range(NBLOCKS):
                stage_list.append(("head", blk))
                for k in range(1, ni):
                    stage_list.append(("iter", blk, k))
                if blk < NBLOCKS - 1:
                    stage_list.append(("tail", blk))
            stage_list.append(("out", 0))
            n_stages = len(stage_list)
            state = [dict() for _ in range(n_tiles)]
            if int(os.environ.get("KERNEL_PREFX", 0)):
                for t in range(n_tiles):
                    state[t]["xt"] = iop.tile(
                        [128, TILE_N], f32r, tag="xt", bufs=n_tiles + 1,
                        name="xt")
                    nc.sync.dma_start(
                        out=state[t]["xt"],
                        in_=r(x_d[:, t * TILE_N:(t + 1) * TILE_N]))
            ramp = int(os.environ.get("KERNEL_RAMP", 1))
            offs = []
            off = 0
            for t in range(n_tiles):
                offs.append(off)
                # tighter spacing for the first/last few tiles trims the
                # pipeline fill/drain at slight extra steady-state pressure
                if ramp and (t < ramp or t >= n_tiles - 1 - ramp):
                    off += max(stagger - 1, 1)
                else:
                    off += stagger
            total_steps = offs[-1] + n_stages
            order_mode = int(os.environ.get("KERNEL_ORDER", 1))
            for step in range(total_steps):
                live = [t for t in range(n_tiles)
                        if 0 <= step - offs[t] < n_stages]
                if order_mode == 1:
                    live = live[::-1]
                elif order_mode == 2:
                    live.sort(key=lambda t: {"iter": 0, "head": 1, "tail": 2,
                                             "in": 3, "nop": 4, "out": 5}[
                        stage_list[step - offs[t]][0]])
                for t in live:
                    si = step - offs[t]
                    st = stage_list[si]
                    s = state[t]
                    if st[0] == "nop":
                        continue
                    if st[0] == "in":
                        if "xt" not in s:
                            s["xt"] = stage_in(t)
                    elif st[0] == "head":
                        s["c"], s["t8"], s["ps"] = emit_head(
                            s["xt"], s.get("h"), st[1])
                    elif st[0] == "iter":
                        blk, k = st[1], st[2]
                        out = emit_iter(s["c"], s["t8"], blk,
                                        last=(k == ni - 1), first=(k == 1),
                                        head_ps=s["ps"], xtf=s["xt"],
                                        h=s.get("h"))
                        if k == ni - 1:
                            s["tf"] = out
                        else:
                            s["t8"] = out
                    elif st[0] == "tail":
                        s["h"] = emit_tail(s["xt"], s.get("h"), s["tf"], st[1])
                    else:
                        stage_out(t, s["h"], s["tf"])

    nc.compile()
    return nc


def _prep_weights(W_init, b_init, Wg1, bg1, Wg2, bg2, W_final, b_final,
                  fp8=FP8):
    import ml_dtypes
    F = np.float64
    f = np.float32
    it_np = ml_dtypes.float8_e4m3 if fp8 else f
    Wi, bi = F(np.asarray(W_init)), F(np.asarray(b_init))
    W1, B1 = F(np.asarray(Wg1)), F(np.asarray(bg1))
    W2, B2 = F(np.asarray(Wg2)), F(np.asarray(bg2))
    Wf, bfin = F(np.asarray(W_final)), F(np.asarray(b_final))

    def feat_major(a):   # [NB, 256, 256] -> [128(kp), NB, 2(kg), 256]
        return np.ascontiguousarray(
            a.astype(f).reshape(NBLOCKS, 2, 128, HIDDEN).transpose(2, 0, 1, 3))

    mn = np.stack([-(W2[b] @ W1[b]) for b in range(NBLOCKS)])
    e = np.stack([B1[b] - B2[b] @ W1[b] for b in range(NBLOCKS)])

    # cumulative fold: h_true = h_raw - fold  (block0 h_raw = Wi^T x)
    folds, fold = [], -bi
    for b in range(NBLOCKS):
        folds.append(fold.copy())
        fold = fold + B2[b]
    e2 = np.stack([e[b] - folds[b] @ W1[b] for b in range(NBLOCKS)])
    b12 = np.stack([B1[b] - folds[b] @ W1[b] for b in range(NBLOCKS)])

    def bias_pack(a):    # [NB, 256] -> [128, NB, 2]
        return np.ascontiguousarray(
            a.astype(f).reshape(NBLOCKS, 2, 128).transpose(2, 0, 1))

    w10 = Wi @ W1[0]                      # [128, 256]
    wft = -(W2[NBLOCKS - 1] @ Wf)         # [256, 128]
    bf2 = bfin - fold @ Wf                # [128]

    e8p = np.zeros((1, NBLOCKS, 2, 2, 128), np.float64)
    for b in range(NBLOCKS):
        for m in range(2):
            v = e2[b, m * 128:(m + 1) * 128]
            v8 = v.astype(np.float32).astype(it_np).astype(np.float64)
            e8p[0, b, m, 0] = v8
            e8p[0, b, m, 1] = (v - v8).astype(np.float32)
    ones8 = np.ones((1, 2, TILE_N), np.float32)

    id8 = np.zeros((128, 2, 128), np.float32)
    for p in range(128):
        id8[p, :, p] = 1.0

    return {
        "id8": np.ascontiguousarray(id8.astype(it_np)),
        **({"e8p": np.ascontiguousarray(e8p.astype(np.float32).astype(it_np)),
            "ones8": np.ascontiguousarray(ones8.astype(it_np))}
           if DELTA else {}),
        "w10": np.ascontiguousarray(w10.astype(f)),
        "w1": feat_major(W1),
        "mn": np.ascontiguousarray(
            mn.astype(f).reshape(NBLOCKS, 2, 128, HIDDEN)
            .transpose(2, 0, 1, 3).astype(it_np)),
        "w2": feat_major(-W2),
        "wi": np.ascontiguousarray(Wi.astype(f)),
        "wfh": np.ascontiguousarray(
            Wf.astype(f).reshape(2, 128, OUT).transpose(1, 0, 2)),
        "wft": np.ascontiguousarray(
            wft.astype(f).reshape(2, 128, OUT).transpose(1, 0, 2)),
        "d": bias_pack(np.stack([B2[b] @ W1[b] for b in range(NBLOCKS)])),
        "e": bias_pack(e2),
        "b1": bias_pack(b12),
        "bf": np.ascontiguousarray(bf2.astype(f).reshape(128, 1)),
    }


def kernel(x, W_init, b_init, Wg1, bg1, Wg2, bg2, W_final, b_final):
    from concourse.bass_utils import run_bass_kernel_spmd

    n_tiles = int(os.environ.get("KERNEL_N_TILES", N_TILES))
    key = ("nc", n_tiles, NI, FP8, PAIR, UNIFY, SPLITR, SPLITT, ENGMODE, STAGGER, PERM, INJ8, ITFUSE, REUSE, RECOMP, DELTA)
    if key not in _CACHE:
        _CACHE[key] = _build(n_tiles, NI, FP8, PAIR)
    nc = _CACHE[key]

    w = _prep_weights(W_init, b_init, Wg1, bg1, Wg2, bg2, W_final, b_final)
    x = np.asarray(x, np.float32).reshape(N_CORES, B_CORE, LATENT)
    in_maps = [dict(w, x=np.ascontiguousarray(x[i].T))
               for i in range(N_CORES)]

    res = run_bass_kernel_spmd(nc, in_maps, core_ids=list(range(N_CORES)))
    y = np.concatenate([np.asarray(res.results[i]["y"]).T
                        for i in range(N_CORES)], axis=0)
    return y.astype(np.float32)

